# revision 6
# baseline (speedup 1.0000x reference)
"""Trainium2 Bass kernel for nn_AttnBias (Graphormer-style attention bias).

out[b,h,n,m] = spatial_emb[spatial_pos[b,n,m], h]
             + (1/path_len) * sum_k sum_d edge_feat_emb[edge_input[b,n,m,k], d]
                                         * edge_pos_emb.reshape(K,D,H)[k,d,h]

Strategy (8 NeuronCores, pure data parallel over (b, n)):
  - Algebraic refactor: combined table T[k,e,h] = sum_d E[e,d] * W[k,d,h]
    (32*1025*8 f32 ~ 1MB). Rows with e==0 are zero (padding), so the k-sum
    needs no masking; only path_len needs the e!=0 count.
  - The gather runs on-device via the GPSIMD ap_gather extended instruction:
    per 16-partition group, an int16 index list gathers rows of per-channel
    SBUF tables. Channels 0..7 hold the 8 h-slices of T for 16 hops
    (k split into two halves so indices fit int16), channel 8 holds a 0/1
    mask table so the same gather also produces the path-length counts.
  - DVE reduces over hops at line rate; path_len is broadcast to the h
    channels with small SBUF->SBUF DMAs; reciprocal-multiply; the spatial
    bias is gathered the same way from a 512-row table and added.
  - Host work is limited to sharding/marshalling: dtype conversion
    (int64 -> int16 indices; Trainium has no int64), packing the wrapped
    per-group index lists, and building the replicated lookup tables.
"""
import sys

sys.path.insert(0, "/opt/trn_rl_repo")

import numpy as np

import concourse.bass as bass  # noqa: F401  (bass types used indirectly)
import concourse.mybir as mybir
import concourse.tile as tile
import concourse.bacc as bacc
from concourse.bass_utils import run_bass_kernel_spmd

# Problem constants (hardcoded per contract)
B, N, K, D, H = 4, 256, 32, 32, 8
NUM_SPATIAL, NUM_EDGES = 512, 1024
E_ROWS = NUM_EDGES + 1            # 1025
KH = K // 2                       # 16 hops per table half
V_EDGE = KH * E_ROWS              # 16400 rows per half-table
V_SPAT = NUM_SPATIAL              # 512
P = 128
N_CORES = 8
ROWS_PER_CORE = 128               # (b, n) rows per core
PAIRS_PER_GROUP = 16 * N          # 4096 (16 n-rows x 256 m)
CHUNK_IDX = 8192                  # ap_gather indices per call
PAIRS_PER_CHUNK = CHUNK_IDX // KH  # 512
N_CHUNKS = PAIRS_PER_GROUP // PAIRS_PER_CHUNK  # 8


def _build_nc(extra_rounds=0):
    """extra_rounds re-executes the (idempotent) edge gather phase for
    delta-based timing; results are unchanged."""
    nc = bacc.Bacc("TRN2", target_bir_lowering=False, debug=False,
                   num_devices=N_CORES)
    t_tab_a = nc.dram_tensor("tab_a", [16, V_EDGE], mybir.dt.float32,
                             kind="ExternalInput")
    t_tab_b = nc.dram_tensor("tab_b", [16, V_EDGE], mybir.dt.float32,
                             kind="ExternalInput")
    t_tab_s = nc.dram_tensor("tab_s", [16, V_SPAT], mybir.dt.float32,
                             kind="ExternalInput")
    t_idx_a = nc.dram_tensor("idx_a", [P, PAIRS_PER_GROUP], mybir.dt.int16,
                             kind="ExternalInput")
    t_idx_b = nc.dram_tensor("idx_b", [P, PAIRS_PER_GROUP], mybir.dt.int16,
                             kind="ExternalInput")
    t_idx_s = nc.dram_tensor("idx_s", [P, PAIRS_PER_GROUP // 16], mybir.dt.int16,
                             kind="ExternalInput")
    t_out = nc.dram_tensor("out", [H, ROWS_PER_CORE, N], mybir.dt.float32,
                           kind="ExternalOutput")

    with tile.TileContext(nc) as tc, tc.tile_pool(name="sbuf", bufs=1) as pool:
        tab = pool.tile([P, V_EDGE], mybir.dt.float32, name="tab")
        tab_s = pool.tile([P, V_SPAT], mybir.dt.float32, name="tab_s")
        g_sb = pool.tile([P, CHUNK_IDX], mybir.dt.float32, name="g_sb")
        red = pool.tile([P, PAIRS_PER_GROUP], mybir.dt.float32, name="red")
        tmp = pool.tile([P, PAIRS_PER_CHUNK], mybir.dt.float32, name="tmp")
        spat = pool.tile([P, PAIRS_PER_GROUP], mybir.dt.float32, name="spat")
        lb = pool.tile([P, PAIRS_PER_GROUP], mybir.dt.float32, name="lb")
        idx_a = pool.tile([P, PAIRS_PER_GROUP], mybir.dt.int16, name="idx_a")
        idx_b = pool.tile([P, PAIRS_PER_GROUP], mybir.dt.int16, name="idx_b")
        idx_s = pool.tile([P, PAIRS_PER_GROUP // 16], mybir.dt.int16, name="idx_s")

        nc.sync.dma_start(out=idx_a[:], in_=t_idx_a[:])
        nc.sync.dma_start(out=idx_b[:], in_=t_idx_b[:])
        nc.sync.dma_start(out=idx_s[:], in_=t_idx_s[:])

        for _round in range(1 + extra_rounds):
            # tab_a/tab_b are [16, V]; replicate the 16-channel pattern to
            # all 8 groups. Pass A: hops k=0..15 (+ mask counts on ch 8);
            # the reduce OVERWRITES red, so re-running a round is idempotent.
            for g in range(8):
                nc.sync.dma_start(out=tab[16 * g:16 * (g + 1), :], in_=t_tab_a[:])
            for c in range(N_CHUNKS):
                sl = slice(c * PAIRS_PER_CHUNK * KH // 16,
                           (c + 1) * PAIRS_PER_CHUNK * KH // 16)
                nc.gpsimd.ap_gather(
                    out_ap=g_sb[:], in_ap=tab[:],
                    idxs_ap=idx_a[:, sl],
                    channels=P, num_elems=V_EDGE, d=1, num_idxs=CHUNK_IDX)
                nc.vector.reduce_sum(
                    red[:, c * PAIRS_PER_CHUNK:(c + 1) * PAIRS_PER_CHUNK],
                    g_sb[:].rearrange("p (l j) -> p l j", j=KH),
                    axis=mybir.AxisListType.X)

            # Pass B: hops k=16..31, accumulated into red
            for g in range(8):
                nc.sync.dma_start(out=tab[16 * g:16 * (g + 1), :], in_=t_tab_b[:])
            for c in range(N_CHUNKS):
                sl = slice(c * PAIRS_PER_CHUNK * KH // 16,
                           (c + 1) * PAIRS_PER_CHUNK * KH // 16)
                nc.gpsimd.ap_gather(
                    out_ap=g_sb[:], in_ap=tab[:],
                    idxs_ap=idx_b[:, sl],
                    channels=P, num_elems=V_EDGE, d=1, num_idxs=CHUNK_IDX)
                nc.vector.reduce_sum(
                    tmp[:],
                    g_sb[:].rearrange("p (l j) -> p l j", j=KH),
                    axis=mybir.AxisListType.X)
                csl = slice(c * PAIRS_PER_CHUNK, (c + 1) * PAIRS_PER_CHUNK)
                nc.vector.tensor_add(red[:, csl], red[:, csl], tmp[:])

        # Spatial gather
        for g in range(8):
            nc.sync.dma_start(out=tab_s[16 * g:16 * (g + 1), :], in_=t_tab_s[:])
        nc.gpsimd.ap_gather(
            out_ap=spat[:], in_ap=tab_s[:], idxs_ap=idx_s[:],
            channels=P, num_elems=V_SPAT, d=1, num_idxs=PAIRS_PER_GROUP)

        # Broadcast the path-length counts (channel 8 of each group) to the
        # h channels 0..7, clamp to >=1, reciprocal, scale, add spatial.
        nc.vector.memset(lb[:], 1.0)
        for g in range(8):
            for ch in range(8):
                nc.sync.dma_start(out=lb[16 * g + ch:16 * g + ch + 1, :],
                                  in_=red[16 * g + 8:16 * g + 9, :])
        nc.vector.tensor_scalar_max(lb[:], lb[:], 1.0)
        nc.vector.reciprocal(lb[:], lb[:])
        nc.vector.tensor_mul(red[:], red[:], lb[:])
        nc.vector.tensor_add(red[:], red[:], spat[:])

        # Store: channel 16g+h holds pairs of group g (n rows 16g..16g+15).
        # out[h, nl, m]: group g's block is out[h, 16g:16(g+1), :].
        out_flat = t_out[:].rearrange("h r m -> h (r m)")
        for h in range(H):
            for g in range(8):
                nc.sync.dma_start(
                    out=out_flat[h, g * PAIRS_PER_GROUP:(g + 1) * PAIRS_PER_GROUP],
                    in_=red[16 * g + h:16 * g + h + 1, :])
    nc.compile()
    return nc


_NC_CACHE = None


def _get_nc():
    global _NC_CACHE
    if _NC_CACHE is None:
        _NC_CACHE = _build_nc()
    return _NC_CACHE


def _host_prep(spatial_pos, edge_input, spatial_emb, edge_feat_emb, edge_pos_emb):
    """Build the per-core input maps."""
    # Combined table T[k, e, h] = sum_d E[e, d] * W[k, d, h]
    W = np.asarray(edge_pos_emb, np.float32).reshape(K, D, H)
    E = np.asarray(edge_feat_emb, np.float32)
    T = np.einsum("ed,kdh->keh", E, W).astype(np.float32)  # [32, 1025, 8]

    # Half-table channel patterns [16, V_EDGE]:
    # rows 0..7 = h-slices (row-major over (j, e)), row 8 = valid-hop mask.
    def half_pattern(Th):  # Th: [16, 1025, 8]
        pat = np.zeros((16, V_EDGE), np.float32)
        pat[:8] = Th.transpose(2, 0, 1).reshape(8, V_EDGE)
        mask = (np.arange(V_EDGE) % E_ROWS != 0).astype(np.float32)
        pat[8] = mask
        return pat

    tab_a = half_pattern(T[:KH])
    tab_b = half_pattern(T[KH:])
    tab_s = np.zeros((16, V_SPAT), np.float32)
    tab_s[:8] = np.asarray(spatial_emb, np.float32).T  # [8, 512]

    j_off = (np.arange(KH, dtype=np.int16) * E_ROWS)  # [16]

    in_maps = []
    for core in range(N_CORES):
        b = core // 2
        n0 = (core % 2) * ROWS_PER_CORE
        # edge slice [128 rows, 256 m, 32 k] -> int16
        e16 = np.asarray(edge_input[b, n0:n0 + ROWS_PER_CORE], np.int16)
        s16 = np.asarray(spatial_pos[b, n0:n0 + ROWS_PER_CORE], np.int16)

        # idx_a[16g + j, pl] = j*1025 + e[16g + pl//256, pl%256, j]
        # e16 view: [8 g, 16 nl, 256 m, 32 k]
        eg = e16.reshape(8, 16, N, K)
        # [8, 16j, 16nl*256m] for each half
        ia = (eg[..., :KH].astype(np.int16)
              + j_off[None, None, None, :]).transpose(0, 3, 1, 2)
        idx_a = ia.reshape(8 * KH, PAIRS_PER_GROUP).astype(np.int16)
        ib = (eg[..., KH:].astype(np.int16)
              + j_off[None, None, None, :]).transpose(0, 3, 1, 2)
        idx_b = ib.reshape(8 * KH, PAIRS_PER_GROUP).astype(np.int16)

        # idx_s: per group, 4096 indices wrapped: [16 w, 256 s] with
        # flat i = s*16 + w -> pair pl = i
        sg = s16.reshape(8, PAIRS_PER_GROUP)  # [g, pl]
        idx_s = sg.reshape(8, PAIRS_PER_GROUP // 16, 16).transpose(0, 2, 1) \
                  .reshape(P, PAIRS_PER_GROUP // 16).astype(np.int16)

        in_maps.append({
            "tab_a": tab_a, "tab_b": tab_b, "tab_s": tab_s,
            "idx_a": idx_a, "idx_b": idx_b, "idx_s": idx_s,
        })
    return in_maps


def kernel(spatial_pos, edge_input, spatial_emb, edge_feat_emb, edge_pos_emb):
    nc = _get_nc()
    in_maps = _host_prep(spatial_pos, edge_input, spatial_emb,
                         edge_feat_emb, edge_pos_emb)
    res = run_bass_kernel_spmd(nc, in_maps, core_ids=list(range(N_CORES)))
    out = np.empty((B, H, N, N), np.float32)
    for core in range(N_CORES):
        b = core // 2
        n0 = (core % 2) * ROWS_PER_CORE
        out[b, :, n0:n0 + ROWS_PER_CORE, :] = res.results[core]["out"]
    return out


if __name__ == "__main__":
    # quick self-check against a local numpy reference
    rng = np.random.default_rng(0)
    spatial_pos = rng.integers(0, NUM_SPATIAL, (B, N, N)).astype(np.int64)
    edge_input = rng.integers(0, NUM_EDGES + 1, (B, N, N, K)).astype(np.int64)
    spatial_emb = rng.standard_normal((NUM_SPATIAL, H)).astype(np.float32)
    spatial_emb[0] = 0
    edge_feat_emb = rng.standard_normal((NUM_EDGES + 1, D)).astype(np.float32)
    edge_feat_emb[0] = 0
    edge_pos_emb = rng.standard_normal((K, D * H)).astype(np.float32)

    got = kernel(spatial_pos, edge_input, spatial_emb, edge_feat_emb,
                 edge_pos_emb)

    Wr = edge_pos_emb.reshape(K, D, H)
    ef = edge_feat_emb[edge_input]
    terms = np.einsum("bnmkd,kdh->bnmkh", ef, Wr)
    maskr = edge_input != 0
    plen = np.maximum(maskr.sum(-1, keepdims=True), 1.0)
    eb = terms.sum(-2) / plen
    ref = spatial_emb[spatial_pos].transpose(0, 3, 1, 2) \
        + eb.transpose(0, 3, 1, 2)
    err = np.abs(got - ref).max() / (np.abs(ref).max() + 1e-9)
    print("rel err:", err)


# revision 7
# speedup vs baseline: 1.0563x; 1.0563x over previous
"""Trainium2 Bass kernel for nn_AttnBias (Graphormer-style attention bias).

out[b,h,n,m] = spatial_emb[spatial_pos[b,n,m], h]
             + (1/path_len) * sum_k sum_d edge_feat_emb[edge_input[b,n,m,k], d]
                                         * edge_pos_emb.reshape(K,D,H)[k,d,h]

Strategy (8 NeuronCores, pure data parallel over (b, n)):
  - Algebraic refactor: combined table T[k,e,h] = sum_d E[e,d] * W[k,d,h]
    (32*1025*8 f32 ~ 1MB). Rows with e==0 are zero (padding), so the k-sum
    needs no masking; only path_len needs the e!=0 count.
  - The gather runs on-device via the GPSIMD ap_gather extended instruction:
    per 16-partition group, an int16 index list gathers rows of per-channel
    SBUF tables. Channels 0..7 hold the 8 h-slices of T for 16 hops
    (k split into two halves so indices fit int16), channel 8 holds a 0/1
    mask table so the same gather also produces the path-length counts.
  - DVE reduces over hops at line rate; path_len is broadcast to the h
    channels with small SBUF->SBUF DMAs; reciprocal-multiply; the spatial
    bias is gathered the same way from a 512-row table and added.
  - Host work is limited to sharding/marshalling: dtype conversion
    (int64 -> int16 indices; Trainium has no int64), packing the wrapped
    per-group index lists, and building the replicated lookup tables.
"""
import sys

sys.path.insert(0, "/opt/trn_rl_repo")

import numpy as np

import concourse.bass as bass  # noqa: F401  (bass types used indirectly)
import concourse.mybir as mybir
import concourse.tile as tile
import concourse.bacc as bacc
from concourse.bass_utils import run_bass_kernel_spmd

# Problem constants (hardcoded per contract)
B, N, K, D, H = 4, 256, 32, 32, 8
NUM_SPATIAL, NUM_EDGES = 512, 1024
E_ROWS = NUM_EDGES + 1            # 1025
KH = K // 2                       # 16 hops per table half
V_EDGE = KH * E_ROWS              # 16400 rows per half-table
V_SPAT = NUM_SPATIAL              # 512
P = 128
N_CORES = 8
ROWS_PER_CORE = 128               # (b, n) rows per core
PAIRS_PER_GROUP = 16 * N          # 4096 (16 n-rows x 256 m)
CHUNK_IDX = 8192                  # ap_gather indices per call
PAIRS_PER_CHUNK = CHUNK_IDX // KH  # 512
N_CHUNKS = PAIRS_PER_GROUP // PAIRS_PER_CHUNK  # 8


def _build_nc(extra_rounds=0):
    """extra_rounds re-executes the (idempotent) edge gather phase for
    delta-based timing; results are unchanged."""
    nc = bacc.Bacc("TRN2", target_bir_lowering=False, debug=False,
                   num_devices=N_CORES)
    t_tab_a = nc.dram_tensor("tab_a", [16, V_EDGE], mybir.dt.float32,
                             kind="ExternalInput")
    t_tab_b = nc.dram_tensor("tab_b", [16, V_EDGE], mybir.dt.float32,
                             kind="ExternalInput")
    t_tab_s = nc.dram_tensor("tab_s", [16, V_SPAT], mybir.dt.float32,
                             kind="ExternalInput")
    t_idx_a = nc.dram_tensor("idx_a", [P, PAIRS_PER_GROUP], mybir.dt.int16,
                             kind="ExternalInput")
    t_idx_b = nc.dram_tensor("idx_b", [P, PAIRS_PER_GROUP], mybir.dt.int16,
                             kind="ExternalInput")
    t_idx_s = nc.dram_tensor("idx_s", [P, PAIRS_PER_GROUP // 16], mybir.dt.int16,
                             kind="ExternalInput")
    t_out = nc.dram_tensor("out", [H, ROWS_PER_CORE, N], mybir.dt.float32,
                           kind="ExternalOutput")

    with tile.TileContext(nc) as tc, tc.tile_pool(name="sbuf", bufs=1) as pool:
        tab = pool.tile([P, V_EDGE], mybir.dt.float32, name="tab")
        tab_s = pool.tile([P, V_SPAT], mybir.dt.float32, name="tab_s")
        g_sb = pool.tile([P, CHUNK_IDX], mybir.dt.float32, name="g_sb")
        red = pool.tile([P, PAIRS_PER_GROUP], mybir.dt.float32, name="red")
        tmp = pool.tile([P, PAIRS_PER_CHUNK], mybir.dt.float32, name="tmp")
        spat = pool.tile([P, PAIRS_PER_GROUP], mybir.dt.float32, name="spat")
        lb = pool.tile([P, PAIRS_PER_GROUP], mybir.dt.float32, name="lb")
        idx_a = pool.tile([P, PAIRS_PER_GROUP], mybir.dt.int16, name="idx_a")
        idx_b = pool.tile([P, PAIRS_PER_GROUP], mybir.dt.int16, name="idx_b")
        idx_s = pool.tile([P, PAIRS_PER_GROUP // 16], mybir.dt.int16, name="idx_s")

        nc.sync.dma_start(out=idx_a[:], in_=t_idx_a[:])
        nc.sync.dma_start(out=idx_b[:], in_=t_idx_b[:])
        nc.sync.dma_start(out=idx_s[:], in_=t_idx_s[:])

        for _round in range(1 + extra_rounds):
            # tab_a/tab_b are [16, V]; replicate the 16-channel pattern to
            # all 8 groups. Pass A: hops k=0..15 (+ mask counts on ch 8);
            # the reduce OVERWRITES red, so re-running a round is idempotent.
            for g in range(8):
                nc.sync.dma_start(out=tab[16 * g:16 * (g + 1), :], in_=t_tab_a[:])
            for c in range(N_CHUNKS):
                sl = slice(c * PAIRS_PER_CHUNK * KH // 16,
                           (c + 1) * PAIRS_PER_CHUNK * KH // 16)
                nc.gpsimd.ap_gather(
                    out_ap=g_sb[:], in_ap=tab[:],
                    idxs_ap=idx_a[:, sl],
                    channels=P, num_elems=V_EDGE, d=1, num_idxs=CHUNK_IDX)
                nc.vector.reduce_sum(
                    red[:, c * PAIRS_PER_CHUNK:(c + 1) * PAIRS_PER_CHUNK],
                    g_sb[:].rearrange("p (l j) -> p l j", j=KH),
                    axis=mybir.AxisListType.X)

            # Pass B: hops k=16..31, accumulated into red
            for g in range(8):
                nc.sync.dma_start(out=tab[16 * g:16 * (g + 1), :], in_=t_tab_b[:])
            for c in range(N_CHUNKS):
                sl = slice(c * PAIRS_PER_CHUNK * KH // 16,
                           (c + 1) * PAIRS_PER_CHUNK * KH // 16)
                nc.gpsimd.ap_gather(
                    out_ap=g_sb[:], in_ap=tab[:],
                    idxs_ap=idx_b[:, sl],
                    channels=P, num_elems=V_EDGE, d=1, num_idxs=CHUNK_IDX)
                nc.vector.reduce_sum(
                    tmp[:],
                    g_sb[:].rearrange("p (l j) -> p l j", j=KH),
                    axis=mybir.AxisListType.X)
                csl = slice(c * PAIRS_PER_CHUNK, (c + 1) * PAIRS_PER_CHUNK)
                nc.vector.tensor_add(red[:, csl], red[:, csl], tmp[:])

        # Spatial gather
        for g in range(8):
            nc.sync.dma_start(out=tab_s[16 * g:16 * (g + 1), :], in_=t_tab_s[:])
        nc.gpsimd.ap_gather(
            out_ap=spat[:], in_ap=tab_s[:], idxs_ap=idx_s[:],
            channels=P, num_elems=V_SPAT, d=1, num_idxs=PAIRS_PER_GROUP)

        # Broadcast the path-length counts (channel 8 of each group) to the
        # h channels 0..7, clamp to >=1, reciprocal, scale, add spatial.
        nc.vector.memset(lb[:], 1.0)
        for g in range(8):
            for ch in range(8):
                nc.sync.dma_start(out=lb[16 * g + ch:16 * g + ch + 1, :],
                                  in_=red[16 * g + 8:16 * g + 9, :])
        nc.vector.tensor_scalar_max(lb[:], lb[:], 1.0)
        nc.vector.reciprocal(lb[:], lb[:])
        nc.vector.tensor_mul(red[:], red[:], lb[:])
        nc.vector.tensor_add(red[:], red[:], spat[:])

        # Store: channel 16g+h holds pairs of group g (n rows 16g..16g+15).
        # out[h, nl, m]: group g's block is out[h, 16g:16(g+1), :].
        out_flat = t_out[:].rearrange("h r m -> h (r m)")
        for h in range(H):
            for g in range(8):
                nc.sync.dma_start(
                    out=out_flat[h, g * PAIRS_PER_GROUP:(g + 1) * PAIRS_PER_GROUP],
                    in_=red[16 * g + h:16 * g + h + 1, :])
    nc.compile()
    return nc


_NC_CACHE = None


def _get_nc():
    global _NC_CACHE
    if _NC_CACHE is None:
        _NC_CACHE = _build_nc()
    return _NC_CACHE


def _host_prep(spatial_pos, edge_input, spatial_emb, edge_feat_emb, edge_pos_emb):
    """Build the per-core input maps."""
    # Combined table T[k, e, h] = sum_d E[e, d] * W[k, d, h]
    W = np.asarray(edge_pos_emb, np.float32).reshape(K, D, H)
    E = np.asarray(edge_feat_emb, np.float32)
    T = np.einsum("ed,kdh->keh", E, W).astype(np.float32)  # [32, 1025, 8]

    # Half-table channel patterns [16, V_EDGE]:
    # rows 0..7 = h-slices (row-major over (j, e)), row 8 = valid-hop mask.
    def half_pattern(Th):  # Th: [16, 1025, 8]
        pat = np.zeros((16, V_EDGE), np.float32)
        pat[:8] = Th.transpose(2, 0, 1).reshape(8, V_EDGE)
        mask = (np.arange(V_EDGE) % E_ROWS != 0).astype(np.float32)
        pat[8] = mask
        return pat

    tab_a = half_pattern(T[:KH])
    tab_b = half_pattern(T[KH:])
    tab_s = np.zeros((16, V_SPAT), np.float32)
    tab_s[:8] = np.asarray(spatial_emb, np.float32).T  # [8, 512]

    j_off = (np.arange(KH, dtype=np.int16) * E_ROWS)  # [16]

    in_maps = []
    for core in range(N_CORES):
        b = core // 2
        n0 = (core % 2) * ROWS_PER_CORE
        # edge slice [128 rows, 256 m, 32 k] -> int16
        e16 = np.asarray(edge_input[b, n0:n0 + ROWS_PER_CORE], np.int16)
        s16 = np.asarray(spatial_pos[b, n0:n0 + ROWS_PER_CORE], np.int16)

        # idx_a[16g + j, pl] = j*1025 + e[16g + pl//256, pl%256, j]
        # e16 view: [8 g, 16 nl, 256 m, 32 k]
        eg = e16.reshape(8, 16, N, K)
        # [8, 16j, 16nl*256m] for each half
        ia = (eg[..., :KH].astype(np.int16)
              + j_off[None, None, None, :]).transpose(0, 3, 1, 2)
        idx_a = ia.reshape(8 * KH, PAIRS_PER_GROUP).astype(np.int16)
        ib = (eg[..., KH:].astype(np.int16)
              + j_off[None, None, None, :]).transpose(0, 3, 1, 2)
        idx_b = ib.reshape(8 * KH, PAIRS_PER_GROUP).astype(np.int16)

        # idx_s: per group, 4096 indices wrapped: [16 w, 256 s] with
        # flat i = s*16 + w -> pair pl = i
        sg = s16.reshape(8, PAIRS_PER_GROUP)  # [g, pl]
        idx_s = sg.reshape(8, PAIRS_PER_GROUP // 16, 16).transpose(0, 2, 1) \
                  .reshape(P, PAIRS_PER_GROUP // 16).astype(np.int16)

        in_maps.append({
            "tab_a": tab_a, "tab_b": tab_b, "tab_s": tab_s,
            "idx_a": idx_a, "idx_b": idx_b, "idx_s": idx_s,
        })
    return in_maps


def kernel(spatial_pos, edge_input, spatial_emb, edge_feat_emb, edge_pos_emb):
    nc = _get_nc()
    in_maps = _host_prep(spatial_pos, edge_input, spatial_emb,
                         edge_feat_emb, edge_pos_emb)
    try:
        res = run_bass_kernel_spmd(nc, in_maps, core_ids=list(range(N_CORES)))
    except Exception:
        # one retry: a previously wedged NeuronCore usually recovers on the
        # next claim
        import time
        time.sleep(5)
        res = run_bass_kernel_spmd(nc, in_maps, core_ids=list(range(N_CORES)))
    out = np.empty((B, H, N, N), np.float32)
    for core in range(N_CORES):
        b = core // 2
        n0 = (core % 2) * ROWS_PER_CORE
        out[b, :, n0:n0 + ROWS_PER_CORE, :] = res.results[core]["out"]
    return out


if __name__ == "__main__":
    # quick self-check against a local numpy reference
    rng = np.random.default_rng(0)
    spatial_pos = rng.integers(0, NUM_SPATIAL, (B, N, N)).astype(np.int64)
    edge_input = rng.integers(0, NUM_EDGES + 1, (B, N, N, K)).astype(np.int64)
    spatial_emb = rng.standard_normal((NUM_SPATIAL, H)).astype(np.float32)
    spatial_emb[0] = 0
    edge_feat_emb = rng.standard_normal((NUM_EDGES + 1, D)).astype(np.float32)
    edge_feat_emb[0] = 0
    edge_pos_emb = rng.standard_normal((K, D * H)).astype(np.float32)

    got = kernel(spatial_pos, edge_input, spatial_emb, edge_feat_emb,
                 edge_pos_emb)

    Wr = edge_pos_emb.reshape(K, D, H)
    ef = edge_feat_emb[edge_input]
    terms = np.einsum("bnmkd,kdh->bnmkh", ef, Wr)
    maskr = edge_input != 0
    plen = np.maximum(maskr.sum(-1, keepdims=True), 1.0)
    eb = terms.sum(-2) / plen
    ref = spatial_emb[spatial_pos].transpose(0, 3, 1, 2) \
        + eb.transpose(0, 3, 1, 2)
    err = np.abs(got - ref).max() / (np.abs(ref).max() + 1e-9)
    print("rel err:", err)
